# revision 6
# baseline (speedup 1.0000x reference)
"""Bahdanau attention Trainium2 kernel (v2).

Problem shapes (fixed): B=64, T=1024, KS=QS=H=1024, fp32 in/out.
  proj_keys = keys @ W_key                  [B,T,H]
  q         = query @ W_query               [B,1,H]
  scores    = tanh(q + proj_keys) . w_score [B,T]
  alphas    = softmax(mask(scores))         [B,1,T]
  context   = alphas @ values               [B,1,KS]

Sharding: data-parallel over batch across 8 NeuronCores (8 batches/core),
weights replicated.

Host-side prep (layout/dtype only, no FLOPs): keys are pre-transposed to
keysT [KS,T] per batch and all matmul operands are pre-cast to bf16 so the
device does zero transposes and zero dtype-conversion copies.  PSUM
accumulation stays fp32; softmax and outputs are fp32.

Per-core dataflow (per batch):
  - DMA keysT [k,t] and values [t,d] chunks straight into bf16 SBUF tiles
  - projT[h,t] = sum_k W_key[k,h]*keysT[k,t]: stationary = W_key tile
    (bf16, FWL), moving = keysT, one [128,512] PSUM half per (m,n)
  - ScalarE: S = tanh(projT + q[h]) fused PSUM->SBUF (bf16) with
    per-partition bias
  - scores[1,T] = w_score^T @ S: 8 M=1 matmuls issued as 2 rounds of 4
    back-to-back col-tiled (tile_position=(0,32j)) so they run concurrently
    in the PE array; 4 partial rows on partitions {0,32,64,96} summed on DVE
  - softmax on one partition row: reduce_max(negated) -> Exp activation with
    bias=-max and accum_out=sum -> reciprocal -> scale
  - alphas row -> columns via 8 tiny PE transposes
  - context[1,KS] = sum_t alphas[t]*values[t,:]: col-tiled like scores
  - batch b's softmax/alpha-transpose/context are emitted inside batch
    b+1's m-loop so the PE never waits on the softmax chain
"""

import numpy as np

import concourse.bass as bass
import concourse.mybir as mybir
import concourse.tile as tile
from concourse.masks import make_identity

f32 = mybir.dt.float32
bf16 = mybir.dt.bfloat16

P = 128        # partitions
TB = 8         # batches per core
T = 1024       # sequence length
H = 1024       # hidden (= KS = QS)
NC_ = 8        # chunks of 128 along T/H/KS
NH = 512       # matmul moving free-dim (one PSUM bank of fp32)

AX = mybir.AxisListType
ALU = mybir.AluOpType
ACT = mybir.ActivationFunctionType


def _split_drain_waits(nc, max_waits: int = 1):
    """walrus CTRL encoding supports a limited number of sem waits per
    instruction; Tile's final drain can carry many.  Hoist extras onto
    preceding single-wait drains."""
    for func in nc.m.functions:
        for blk in func.blocks:
            new_insts = []
            for inst in blk.instructions:
                si = inst.sync_info
                if si is not None and si.on_wait and len(si.on_wait) > max_waits:
                    waits = list(si.on_wait)
                    extra, keep = waits[:-max_waits], waits[-max_waits:]
                    for j, w in enumerate(extra):
                        new_insts.append(
                            mybir.InstDrain(
                                name=f"{inst.name}-presplit{j}",
                                engine=inst.engine,
                                sync_info=mybir.SyncInfo(on_wait=[w], on_update=[]),
                            )
                        )
                    si.on_wait = keep
                new_insts.append(inst)
            blk.instructions = new_insts


def build_bahdanau_nc(split_drains=True, reps=1, big_io=True):
    """Build the per-core Bass program (identical on all 8 cores)."""
    import contextlib

    nc = bass.Bass(trn_type="TRN2", target_bir_lowering=False, debug=False)

    big = "ExternalInput" if big_io else "Internal"
    # host-pretransposed keys^T per batch: [KS, T] bf16
    keyst_d = nc.dram_tensor("keyst", [TB, H, T], bf16, kind=big).ap()
    values_d = nc.dram_tensor("values", [TB, T, H], bf16, kind=big).ap()
    # weights host-swizzled to [P, kchunk, H] bf16
    wkey_d = nc.dram_tensor("wkey", [P, NC_, H], bf16, kind=big).ap()
    wquery_d = nc.dram_tensor("wquery", [P, NC_, H], bf16, kind=big).ap()
    # queryt: host-prearranged query^T as [p, kchunk, b] bf16
    qtin_d = nc.dram_tensor("qtin", [P, NC_, TB], bf16, kind="ExternalInput").ap()
    # w_score host-prearranged as [p, kchunk] bf16
    wsc_d = nc.dram_tensor("wsc", [P, NC_], bf16, kind="ExternalInput").ap()
    # additive mask bias (0 where visible, -1e30 where masked) fp32
    maskb_d = nc.dram_tensor("maskb", [TB, T], f32, kind="ExternalInput").ap()

    ctx_d = nc.dram_tensor("ctx", [TB, H], f32, kind="ExternalOutput").ap()
    alph_d = nc.dram_tensor("alph", [TB, T], f32, kind="ExternalOutput").ap()

    with tile.TileContext(nc) as tc, contextlib.ExitStack() as ctx:
        # ---- pools
        const_pool = ctx.enter_context(tc.tile_pool(name="const", bufs=1))
        ktr_pool = ctx.enter_context(tc.tile_pool(name="ktr", bufs=2))
        v_pool = ctx.enter_context(tc.tile_pool(name="vpool", bufs=2))
        s_pool = ctx.enter_context(tc.tile_pool(name="spool", bufs=10))
        row_pool = ctx.enter_context(tc.tile_pool(name="rows", bufs=3))
        small_pool = ctx.enter_context(tc.tile_pool(name="small", bufs=2))

        ps_pool = ctx.enter_context(tc.tile_pool(name="psS", bufs=3, space="PSUM"))
        big_ps = ctx.enter_context(tc.tile_pool(name="bigps", bufs=2, space="PSUM"))
        sm_psum = ctx.enter_context(tc.tile_pool(name="smps", bufs=1, space="PSUM"))

        # ---- preamble
        ident = const_pool.tile([P, P], f32, tag="ident", name="ident")
        make_identity(nc, ident[:, :])

        # prefetch ACT tables for Tanh/Exp during startup DMAs
        warm = const_pool.tile([1, 1], f32, tag="warm", name="warm")
        nc.scalar.activation(warm[:, :], ident[0:1, 0:1], ACT.Tanh)
        nc.scalar.activation(warm[:, :], ident[0:1, 0:1], ACT.Exp)

        wk = const_pool.tile([P, NC_, H], bf16, tag="wk", name="wk")
        nc.sync.dma_start(wk[:, :, :], wkey_d[:, :, :])
        wq = const_pool.tile([P, NC_, H], bf16, tag="wq", name="wq")
        nc.sync.dma_start(wq[:, :, :], wquery_d[:, :, :])
        qtin = const_pool.tile([P, NC_, TB], bf16, tag="qtin", name="qtin")
        nc.sync.dma_start(qtin[:, :, :], qtin_d[:, :, :])
        wsc = const_pool.tile([P, NC_], bf16, tag="wsc", name="wsc")
        nc.sync.dma_start(wsc[:, :], wsc_d[:, :])
        qT = const_pool.tile([P, NC_, TB], f32, tag="qT", name="qT")

        def emit_keyst(b, rep):
            kt = ktr_pool.tile([P, NC_, T], bf16, tag="ktr", name=f"ktr_r{rep}b{b}")
            nc.sync.dma_start(
                kt[:, :, :], keyst_d[b].rearrange("(k p) t -> p k t", p=P)
            )
            return kt

        def emit_values(b, rep):
            vt = v_pool.tile([P, NC_, H], bf16, tag="v", name=f"v_r{rep}b{b}")
            nc.sync.dma_start(
                vt[:, :, :], values_d[b].rearrange("(k p) d -> p k d", p=P)
            )
            return vt

        # ---- steady-state batch pipeline (reps>1 repeats for timing only)
        for rep in range(reps):
            ktr_cur = emit_keyst(0, rep)
            mb_cur = small_pool.tile([1, T], f32, tag="mb", name=f"mb_r{rep}b0")
            nc.sync.dma_start(mb_cur[:, :], maskb_d[0:1, :])
            v_cur = emit_values(0, rep)

            if rep == 0:
                # q projection: all 64 [h,b] columns accumulate in one PSUM
                # tile; bf16 weights -> FWL weight loads
                psq = sm_psum.tile([P, NC_ * TB], f32, tag="sm", name="psq")
                for m in range(NC_):
                    for k in range(NC_):
                        nc.tensor.matmul(
                            psq[:, m * TB : (m + 1) * TB],
                            lhsT=wq[:, k, m * P : (m + 1) * P],
                            rhs=qtin[:, k, :],
                            start=(k == 0),
                            stop=(k == NC_ - 1),
                        )
                nc.scalar.copy(
                    qT[:, :, :], psq[:, :].rearrange("p (m b) -> p m b", m=NC_)
                )

            # deferred per-batch state
            pend = None  # (b, arow, vt, mb) awaiting paT/ctx emission

            def emit_paT(st):
                b, arow, vt = st
                paT = sm_psum.tile([P, TB], f32, tag="sm", name=f"paT{rep}_{b}")
                for k in range(NC_):
                    nc.tensor.transpose(
                        paT[:, k : k + 1],
                        arow[0:1, k * P : (k + 1) * P],
                        ident[0:1, 0:1],
                    )
                aT = small_pool.tile([P, NC_], bf16, tag="aT", name=f"aT{rep}_{b}")
                nc.vector.tensor_copy(aT[:, :], paT[:, :])
                return aT

            def emit_ctx(st, aT):
                b, arow, vt = st
                pcx = big_ps.tile([P, T], f32, tag="bps", name=f"pcx{rep}_{b}")
                for r in range(2):
                    for n in range(2):
                        for j in range(4):
                            k = 4 * r + j
                            nc.tensor.matmul(
                                pcx[32 * j : 32 * j + 1, n * NH : (n + 1) * NH],
                                lhsT=aT[:, k : k + 1],
                                rhs=vt[:, k, n * NH : (n + 1) * NH],
                                start=(r == 0),
                                stop=(r == 1),
                                tile_position=(0, 32 * j),
                            )
                # cross-base-partition TensorCopy is legal; TensorTensor
                # requires co-based operands and <=1 PSUM operand
                cp = []
                for i, p0 in enumerate((32, 64, 96)):
                    r = row_pool.tile(
                        [1, T], f32, tag="row", bufs=6, name=f"cxc{rep}_{b}_{i}"
                    )
                    nc.vector.tensor_copy(r[:, :], pcx[p0 : p0 + 1, :])
                    cp.append(r)
                t0 = row_pool.tile([1, T], f32, tag="row", bufs=6, name=f"cxa{rep}_{b}")
                nc.vector.tensor_add(t0[:, :], pcx[0:1, :], cp[0][:, :])
                t1 = row_pool.tile([1, T], f32, tag="row", bufs=6, name=f"cxb{rep}_{b}")
                nc.vector.tensor_add(t1[:, :], cp[1][:, :], cp[2][:, :])
                cxr = row_pool.tile([1, T], f32, tag="row", bufs=6, name=f"cxr{rep}_{b}")
                nc.vector.tensor_add(cxr[:, :], t0[:, :], t1[:, :])
                nc.sync.dma_start(ctx_d[b : b + 1, :], cxr[0:1, :H])

            for b in range(TB):
                last = b == TB - 1
                if not last:
                    ktr_next = emit_keyst(b + 1, rep)
                    mb_next = small_pool.tile(
                        [1, T], f32, tag="mb", name=f"mb_r{rep}b{b + 1}"
                    )
                    nc.sync.dma_start(mb_next[:, :], maskb_d[b + 1 : b + 2, :])
                    v_next = emit_values(b + 1, rep)

                # score PSUM: 4 col-tiled partial rows on partitions
                # {0,32,64,96}, n-halves in separate banks
                psc = big_ps.tile([P, T], f32, tag="bps", name=f"psc{rep}_{b}")
                ss = []  # tanh tiles of this batch
                for m in range(NC_):
                    ps = [
                        ps_pool.tile([P, NH], f32, tag="ps", name=f"ps{rep}_{b}_{m}_{n}")
                        for n in range(2)
                    ]
                    for k in range(NC_):
                        for n in range(2):
                            nc.tensor.matmul(
                                ps[n][:, :],
                                lhsT=wk[:, k, m * P : (m + 1) * P],
                                rhs=ktr_cur[:, k, n * NH : (n + 1) * NH],
                                start=(k == 0),
                                stop=(k == NC_ - 1),
                            )
                    s = s_pool.tile([P, T], bf16, tag="s", name=f"s{rep}_{b}_{m}")
                    for n in range(2):
                        nc.scalar.activation(
                            s[:, n * NH : (n + 1) * NH],
                            ps[n][:, :],
                            ACT.Tanh,
                            bias=qT[:, m, b : b + 1],
                        )
                    ss.append(s)

                    if m == 2 and pend is not None:
                        pend_aT = emit_paT(pend)
                    if m == 3 and pend is not None:
                        emit_ctx(pend, pend_aT)
                        pend = None
                    if m == 3 or m == NC_ - 1:
                        # col-tiled score round: 4 back-to-back M=1 matmuls
                        # run concurrently in distinct col groups
                        r = 0 if m == 3 else 1
                        for n in range(2):
                            for j in range(4):
                                nc.tensor.matmul(
                                    psc[32 * j : 32 * j + 1, n * NH : (n + 1) * NH],
                                    lhsT=wsc[:, 4 * r + j : 4 * r + j + 1],
                                    rhs=ss[4 * r + j][:, n * NH : (n + 1) * NH],
                                    start=(r == 0),
                                    stop=(r == 1),
                                    tile_position=(0, 32 * j),
                                )

                # combine 4 partial score rows + mask bias (DVE, 1 lane);
                # cross-base rows go through TensorCopy first (walrus:
                # TensorTensor needs co-based operands, <=1 PSUM operand)
                scp = []
                for i, p0 in enumerate((32, 64, 96)):
                    r = row_pool.tile(
                        [1, T], f32, tag="row", bufs=6, name=f"scc{rep}_{b}_{i}"
                    )
                    nc.vector.tensor_copy(r[:, :], psc[p0 : p0 + 1, :])
                    scp.append(r)
                t0 = row_pool.tile([1, T], f32, tag="row", bufs=6, name=f"sa{rep}_{b}")
                nc.vector.tensor_add(t0[:, :], psc[0:1, :], scp[0][:, :])
                t1 = row_pool.tile([1, T], f32, tag="row", bufs=6, name=f"sb{rep}_{b}")
                nc.vector.tensor_add(t1[:, :], scp[1][:, :], scp[2][:, :])
                sc = row_pool.tile([1, T], f32, tag="row", bufs=6, name=f"sc{rep}_{b}")
                nc.vector.tensor_add(sc[:, :], t0[:, :], t1[:, :])
                nc.vector.tensor_add(sc[:, :], sc[:, :], mb_cur[:, :])

                # softmax on one partition row
                nmax = small_pool.tile([1, 1], f32, tag="nmax", name=f"nmax{rep}_{b}")
                nc.vector.tensor_reduce(
                    nmax[:, :], sc[:, :], axis=AX.X, op=ALU.max, negate=True
                )
                arow = row_pool.tile([1, T], f32, tag="arow", bufs=2, name=f"arow{rep}_{b}")
                ssum = small_pool.tile([1, 1], f32, tag="ssum", name=f"ssum{rep}_{b}")
                nc.scalar.activation(
                    arow[:, :], sc[:, :], ACT.Exp, bias=nmax[:, :], accum_out=ssum[:, :]
                )
                rinv = small_pool.tile([1, 1], f32, tag="rinv", name=f"rinv{rep}_{b}")
                nc.vector.reciprocal(rinv[:, :], ssum[:, :])
                nc.vector.tensor_scalar_mul(arow[:, :], arow[:, :], rinv[:, :])
                nc.sync.dma_start(alph_d[b : b + 1, :], arow[:, :])

                pend = (b, arow, v_cur)
                if not last:
                    ktr_cur = ktr_next
                    mb_cur = mb_next
                    v_cur = v_next

            # tail: last batch's alpha transpose + context
            aT = emit_paT(pend)
            emit_ctx(pend, aT)

    if split_drains:
        _split_drain_waits(nc)
    return nc


_NC_CACHE = None


def _get_nc():
    global _NC_CACHE
    if _NC_CACHE is None:
        _NC_CACHE = build_bahdanau_nc()
    return _NC_CACHE


def _bf16(a):
    import ml_dtypes

    return np.asarray(a).astype(ml_dtypes.bfloat16)


def make_in_maps(query, mask, values, keys, W_key, W_query, w_score):
    """Shard full inputs into per-core input maps (host-side layout only)."""
    query = np.asarray(query, dtype=np.float32)
    mask = np.asarray(mask)
    values = np.asarray(values, dtype=np.float32)
    keys = np.asarray(keys, dtype=np.float32)
    W_key = np.asarray(W_key, dtype=np.float32)
    W_query = np.asarray(W_query, dtype=np.float32)
    w_score = np.asarray(w_score, dtype=np.float32)

    B = query.shape[0]
    n_cores = B // TB
    maskb = np.where(mask, np.float32(0.0), np.float32(-1e30)).astype(np.float32)
    wsc_in = _bf16(np.ascontiguousarray(w_score.reshape(NC_, P).T))
    wk_in = _bf16(np.ascontiguousarray(W_key.reshape(NC_, P, H).transpose(1, 0, 2)))
    wq_in = _bf16(np.ascontiguousarray(W_query.reshape(NC_, P, H).transpose(1, 0, 2)))
    keyst = _bf16(keys.transpose(0, 2, 1))  # [B, KS, T]
    values_b = _bf16(values)

    in_maps = []
    for c in range(n_cores):
        sl = slice(c * TB, (c + 1) * TB)
        qt = query[sl, 0, :].T  # [QS, TB]
        qtin = _bf16(np.ascontiguousarray(qt.reshape(NC_, P, TB).transpose(1, 0, 2)))
        in_maps.append(
            {
                "keyst": np.ascontiguousarray(keyst[sl]),
                "values": np.ascontiguousarray(values_b[sl]),
                "wkey": wk_in,
                "wquery": wq_in,
                "qtin": qtin,
                "wsc": wsc_in,
                "maskb": np.ascontiguousarray(maskb[sl]),
            }
        )
    return in_maps


def timing_in_maps(n_cores=8):
    """In-maps for big_io=False timing variants (small ExternalInputs only)."""
    import ml_dtypes

    rng = np.random.default_rng(0)
    m = {
        "qtin": rng.standard_normal((P, NC_, TB)).astype(ml_dtypes.bfloat16),
        "wsc": rng.standard_normal((P, NC_)).astype(ml_dtypes.bfloat16),
        "maskb": np.zeros((TB, T), dtype=np.float32),
    }
    return [m] * n_cores


def kernel(query, mask, values, keys, W_key, W_query, w_score):
    from concourse.bass_utils import run_bass_kernel_spmd

    B = np.asarray(query).shape[0]
    n_cores = B // TB
    in_maps = make_in_maps(query, mask, values, keys, W_key, W_query, w_score)
    nc = _get_nc()
    try:
        res = run_bass_kernel_spmd(nc, in_maps, core_ids=list(range(n_cores)))
    except Exception:
        # transient NRT_EXEC_UNIT_UNRECOVERABLE wedges have been observed to
        # clear on retry
        import time as _time

        _time.sleep(2.0)
        res = run_bass_kernel_spmd(nc, in_maps, core_ids=list(range(n_cores)))
    context = np.concatenate([r["ctx"] for r in res.results], axis=0)
    alphas = np.concatenate([r["alph"] for r in res.results], axis=0)
    return context.reshape(B, 1, H), alphas.reshape(B, 1, T)
